# revision 48
# baseline (speedup 1.0000x reference)
"""Trainium2 Bass kernel for nn_BiLSTMWithLM (B=64, T=1024, D_IN=400).

Data-parallel over batch: 8 cores x 8 sequences each. The LSTM scans are
solved by Jacobi fixed-point sweeps (contraction ~0.2/sweep) so every phase
is bulk work instead of a 2048-step latency-bound chain:

  P1: layer-0 input projections (bf16 matmuls, bias folded via an augmented
      ones-row), stored [p, gate, n] in DRAM with b-major cols n = b*T + t.
  S0: K Jacobi sweeps. Per sweep per direction: gates = pre + Whh h_shift
      (identity-preload PSUM + 4 recurrent matmuls per 512-col chunk), one
      sigmoid over all 4 gate slabs (g-gate prescaled x2 on host so
      tanh(g) = 2*sig(2g) - 1), u = 2*si*sg - si on DVE, then the cell
      recurrence c_t = sf_t*c_{t-1} + u_t via the hardware DVE prefix scan
      (tensor_tensor_scan), tanh(c) on ACT, h = so*tanh(c). The backward
      direction runs the same code on negative-stride (reversed) views.
      Sequence boundaries are exact: per-b H buffers carry a zero column so
      the shifted matmul operand never crosses sequences.
  P2: layer-1 input projections from layer-0 output.
  S1: layer-1 sweeps (same as S0).
  P3: head. BN1/linear/BN2 folded on host into LW/LB; computes
      u = tanh(LW @ l1out + LB) and the logit-difference drive
      du = w3s . u + K0 (written as [b, t] — trivial in b-major layout).
  P4: context scan reformulated as a scalar recurrence on the logit diff
      d_t = du_t + g*d_{t-1} - dl*sp(d_{t-1}) + a*d_{t-2} - b*sp(d_{t-2}),
      solved by Jacobi fixed-point iteration (contraction ~0.085/iter);
      lo0 = -softplus(d), lo1 = d - softplus(d).
"""
import os
import sys

sys.path.insert(0, "/opt/trn_rl_repo")

import numpy as np
import ml_dtypes

import concourse.bass as bass
import concourse.bacc as bacc
import concourse.mybir as mybir
from concourse import tile
from concourse.bass_utils import run_bass_kernel_spmd
from concourse.kernels.tile_matmul import matmul_tile_kernel
from contextlib import ExitStack

BF16 = mybir.dt.bfloat16
F32 = mybir.dt.float32
AF = mybir.ActivationFunctionType
OP = mybir.AluOpType

B, D_IN, H = 64, 400, 128
T = int(os.environ.get("KERNEL_T", "1024"))
N_CORES = 8
BL = B // N_CORES          # 8 local sequences
N = T * BL                 # columns, n = b*T + t (b-major)
C = 512 if T >= 512 else T  # sweep chunk columns (1 PSUM bank per gate)
N_SWEEPS = int(os.environ.get("KERNEL_SWEEPS", "3"))
N_JACOBI = 8
EPS = 1e-5


def _bf16(x):
    return np.asarray(x, dtype=ml_dtypes.bfloat16)


def _perm_gates(w):
    # torch gate order i, f, g, o -> slab order i, g, f, o
    i, f, g, o = np.split(np.asarray(w), 4, axis=0)
    return np.concatenate([i, g, f, o], axis=0)


_BUILD_CACHE = {}


def _build():
    key = (T, N_SWEEPS)
    if key in _BUILD_CACHE:
        return _BUILD_CACHE[key]

    nc = bacc.Bacc("TRN2", target_bir_lowering=False, debug=False,
                   num_devices=N_CORES)

    def din(name, shape, dtype):
        return nc.dram_tensor(name, shape, dtype, kind="ExternalInput").ap()

    def dscratch(name, shape, dtype):
        return nc.dram_tensor(name, shape, dtype).ap()

    # inputs
    xk = din("xk", [128, 4, N], BF16)               # aug x, kxn for P1
    w0 = {d: din(f"w0{d}", [128, 4, 512], BF16) for d in "fb"}
    w1 = {d: din(f"w1{d}", [128, 3, 512], BF16) for d in "fb"}
    whh0 = {d: din(f"whh0{d}", [128, 512], BF16) for d in "fb"}
    whh1 = {d: din(f"whh1{d}", [128, 512], BF16) for d in "fb"}
    lwk = din("lwk", [128, 2, 64], BF16)            # LW.T tiled
    lbv = din("lbv", [64, 1], F32)                  # LB bias
    w3s = din("w3s", [64, 1], BF16)                 # head diff vector
    QB = max(T // 128, 1)                           # 128-col q-blocks per b
    P4P = BL * QB                                   # P4 partitions
    coef = din("coef", [P4P, 8], F32)               # [g, -dl, a, -b, K0]
    shm = din("shm", [128, P4P], BF16)              # block-shift matrix (K-padded)
    ident = din("ident", [128, 128], BF16)
    outv = nc.dram_tensor("outv", [N, 2], F32, kind="ExternalOutput").ap()

    # scratch
    proj0 = {d: dscratch(f"proj0{d}", [128, 4, N], BF16) for d in "fb"}
    proj1 = {d: dscratch(f"proj1{d}", [128, 4, N], BF16) for d in "fb"}
    l0out = dscratch("l0out", [128, 3, N], BF16)
    dud = dscratch("dud", [P4P, 128], F32)          # du as [(b q), r]
    DEBUG_TAPS = bool(int(os.environ.get("KERNEL_DEBUG_TAPS", "0")))
    if DEBUG_TAPS:
        hdbg = dscratch("hdbg", [N_SWEEPS, 128, BL, T + 1], BF16)
        udbg = dscratch("udbg", [N_SWEEPS, 128, BL, T], BF16)
        fdbg = dscratch("fdbg", [N_SWEEPS, 128, 2, BL, T], BF16)
        cdbg = dscratch("cdbg", [N_SWEEPS, 128, BL, T], BF16)
    P4_TAPS = bool(int(os.environ.get("KERNEL_P4_TAPS", "0")))
    if P4_TAPS:
        pvdbg = nc.dram_tensor("pvdbg", [N_JACOBI, P4P, 4], F32,
                               kind="ExternalOutput").ap()
        accdbg = nc.dram_tensor("accdbg", [N_JACOBI, P4P, 128], F32,
                                kind="ExternalOutput").ap()

    NCH = T // C

    with tile.TileContext(nc) as tc:
        # ---- init: l0out kb=2 block (ones row at p=0, zeros elsewhere) ----
        with ExitStack() as ctx:
            pool = ctx.enter_context(tc.tile_pool(name="initp", bufs=1))
            ozt = pool.tile([128, 512], BF16)
            nc.vector.memset(ozt[:], 0.0)
            nc.vector.memset(ozt[0:1, :], 1.0)
            for i in range(N // 512):
                nc.sync.dma_start(l0out[:, 2, bass.ts(i, 512)], ozt[:])

        # ---- P1: layer-0 projections ----
        with ExitStack() as ctx:
            for d in "fb":
                matmul_tile_kernel(tc, w0[d][:], xk[:], proj0[d][:])

        # ---- Jacobi sweep layer ----
        def jacobi_layer(layer, projf, projb, whhf_d, whhb_d,
                         flush_to=None):
            """Runs sweeps; returns (HF, HB) SBUF tiles plus the ExitStack
            owning them (caller must close after consuming). All transient
            pools (incl. PSUM) are closed before returning."""
            hctx = ExitStack()
            hpool = hctx.enter_context(tc.tile_pool(name=f"jh{layer}",
                                                    bufs=1))
            # persistent per-layer state (survives until caller closes hctx)
            HF = hpool.tile([128, BL, T + 1], BF16)   # [.,b,0]=0, h at 1..T
            HB = hpool.tile([128, BL, T + 1], BF16)   # [.,b,T]=0, h at 0..T-1
            nc.vector.memset(HF[:], 0.0)
            nc.vector.memset(HB[:], 0.0)

            ctx = ExitStack()
            cpool = ctx.enter_context(tc.tile_pool(name=f"jc{layer}", bufs=1))
            whf = cpool.tile([128, 512], BF16)
            whb = cpool.tile([128, 512], BF16)
            idt = cpool.tile([128, 128], BF16)
            nc.sync.dma_start(whf[:], whhf_d[:])
            nc.sync.dma_start(whb[:], whhb_d[:])
            nc.sync.dma_start(idt[:], ident[:])
            FO = cpool.tile([128, 2, BL, T], BF16)    # sig(f), sig(o)
            U = cpool.tile([128, BL, T], BF16)
            CC = cpool.tile([128, BL, T], BF16)

            ppool = ctx.enter_context(tc.tile_pool(name=f"jp{layer}", bufs=3))
            igpool = ctx.enter_context(tc.tile_pool(name=f"jg{layer}", bufs=3))
            tpool = ctx.enter_context(tc.tile_pool(name=f"jt{layer}", bufs=2))
            psum = ctx.enter_context(
                tc.tile_pool(name=f"jps{layer}", bufs=2, space="PSUM"))

            for k in range(N_SWEEPS):
                for dirf, proj_d, wh in ((True, projf, whf),
                                         (False, projb, whb)):
                    for b in range(BL):
                        for j in range(NCH):
                            jsl = slice(j * C, (j + 1) * C)
                            pf = ppool.tile([128, 4, C], BF16, tag="p")
                            nc.sync.dma_start(
                                pf[:], proj_d[:, :, b * T + j * C:
                                              b * T + (j + 1) * C])
                            ig = igpool.tile([128, 2, C], BF16, tag="ig")
                            if k == 0:
                                # h == 0: gates = sigma(pre), straight from
                                # SBUF -- no PSUM round trip
                                srcs = [pf[:, g, :] for g in range(4)]
                            else:
                                ps = psum.tile([128, 4, C], F32)
                                if dirf:
                                    rhs = HF[:, b, j * C: j * C + C]
                                else:
                                    rhs = HB[:, b, j * C + 1: j * C + C + 1]
                                # HW: an accumulation group is bank-scoped --
                                # pair preload+rec per bank
                                for g in range(4):
                                    nc.tensor.matmul(
                                        ps[:, g, :], idt[:], pf[:, g, :],
                                        start=True, stop=False,
                                        skip_group_check=True)
                                    nc.tensor.matmul(
                                        ps[:, g, :],
                                        wh[:, g * 128:(g + 1) * 128], rhs,
                                        start=False, stop=True,
                                        skip_group_check=True)
                                # HW constraint: an ACT psum read must stay
                                # within one 2KB bank -> one ACT per gate
                                srcs = [ps[:, g, :] for g in range(4)]
                            nc.scalar.activation(ig[:, 0, :], srcs[0],
                                                 AF.Sigmoid)
                            nc.scalar.activation(ig[:, 1, :], srcs[1],
                                                 AF.Sigmoid)
                            nc.scalar.activation(FO[:, 0, b, jsl], srcs[2],
                                                 AF.Sigmoid)
                            nc.scalar.activation(FO[:, 1, b, jsl], srcs[3],
                                                 AF.Sigmoid)
                            t1 = igpool.tile([128, C], BF16, tag="t1")
                            nc.vector.tensor_tensor(
                                t1[:], ig[:, 0, :], ig[:, 1, :], OP.mult)
                            nc.vector.scalar_tensor_tensor(
                                U[:, b, jsl], t1[:], 2.0, ig[:, 0, :],
                                OP.mult, OP.subtract)
                        # per-b tail: scan -> tanh -> h
                        if dirf:
                            nc.vector.tensor_tensor_scan(
                                CC[:, b, :], FO[:, 0, b, :], U[:, b, :],
                                0.0, OP.mult, OP.add)
                        else:
                            nc.vector.tensor_tensor_scan(
                                CC[:, b, ::-1], FO[:, 0, b, ::-1],
                                U[:, b, ::-1], 0.0, OP.mult, OP.add)
                        tcb = tpool.tile([128, T], BF16, tag="tc")
                        nc.scalar.activation(tcb[:], CC[:, b, :], AF.Tanh)
                        hv = (HF[:, b, 1:T + 1] if dirf
                              else HB[:, b, 0:T])
                        nc.vector.tensor_tensor(
                            hv, FO[:, 1, b, :], tcb[:], OP.mult)
                        if flush_to is not None and k == N_SWEEPS - 1:
                            # stream final h per-b so P2's input is mostly
                            # flushed before the layer's tail finishes
                            nc.sync.dma_start(
                                flush_to[:, 0 if dirf else 1,
                                         b * T:(b + 1) * T], hv)
                    if DEBUG_TAPS and layer == 0 and dirf:
                        nc.sync.dma_start(hdbg[k], HF[:])
                        nc.sync.dma_start(udbg[k], U[:])
                        nc.sync.dma_start(fdbg[k], FO[:])
                        nc.sync.dma_start(cdbg[k], CC[:])
            ctx.close()
            return hctx, HF, HB

        # ---- S0 (final h streamed per-b into l0out for P2) ----
        s0ctx, HF0, HB0 = jacobi_layer(0, proj0["f"], proj0["b"],
                                       whh0["f"], whh0["b"],
                                       flush_to=l0out)
        s0ctx.close()

        # ---- P2: layer-1 projections ----
        with ExitStack() as ctx:
            for d in "fb":
                matmul_tile_kernel(tc, w1[d][:], l0out[:], proj1[d][:])

        # ---- S1 ----
        s1ctx, HF1, HB1 = jacobi_layer(1, proj1["f"], proj1["b"],
                                       whh1["f"], whh1["b"])

        # ---- P3: head (reads HF1/HB1 straight from SBUF) ----
        with ExitStack() as ctx:
            cpool = ctx.enter_context(tc.tile_pool(name="headc", bufs=1))
            lw_sb = cpool.tile([128, 2, 64], BF16)
            lb_sb = cpool.tile([64, 1], F32)
            w3_sb = cpool.tile([64, 1], BF16)
            nc.sync.dma_start(lw_sb[:], lwk[:])
            nc.sync.dma_start(lb_sb[:], lbv[:])
            nc.sync.dma_start(w3_sb[:], w3s[:])
            upool = ctx.enter_context(tc.tile_pool(name="headu", bufs=3))
            dpool = ctx.enter_context(tc.tile_pool(name="headd", bufs=3))
            hps = ctx.enter_context(
                tc.tile_pool(name="headps", bufs=2, space="PSUM"))
            hps2 = ctx.enter_context(
                tc.tile_pool(name="headps2", bufs=2, space="PSUM"))
            HC = min(512, T)
            for i in range(N // HC):
                b, jh = i // (T // HC), i % (T // HC)
                jsl = slice(jh * HC, (jh + 1) * HC)
                pu = hps.tile([64, HC], F32)
                nc.tensor.matmul(pu[:], lw_sb[:, 0, :],
                                 HF1[:, b, jh * HC + 1: (jh + 1) * HC + 1],
                                 start=True, stop=False, skip_group_check=True)
                nc.tensor.matmul(pu[:], lw_sb[:, 1, :], HB1[:, b, jsl],
                                 start=False, stop=True, skip_group_check=True)
                ut = upool.tile([64, HC], BF16, tag="u")
                nc.scalar.activation(ut[:], pu[:], AF.Tanh, bias=lb_sb[:])
                pd = hps2.tile([1, HC], F32)
                nc.tensor.matmul(pd[:], w3_sb[:], ut[:])
                dt_ = dpool.tile([1, HC], F32, tag="d")
                nc.vector.tensor_copy(dt_[:], pd[:])
                # dud rows are contiguous in t: write the [1, HC] strip flat
                dst = dud.rearrange("p r -> (p r)")[
                    (b * QB * 128) + jh * HC: (b * QB * 128) + (jh + 1) * HC]
                nc.sync.dma_start(dst.unsqueeze(0), dt_[:])
        s1ctx.close()

        # ---- P4: context solve (jacobi) + output, [(b q), 128] layout ----
        # d[p, r] = d at t = (p % QB)*128 + r of sequence b = p // QB.
        # Cross-block boundary values (r-1, r-2 for r<2) come from row p-1
        # via a PE shift-matrix; shm zeroes rows at sequence starts.
        with ExitStack() as ctx:
            cpool = ctx.enter_context(tc.tile_pool(name="ctxc", bufs=1))
            cf = cpool.tile([P4P, 8], F32)
            sh_sb = cpool.tile([128, P4P], BF16)
            nc.sync.dma_start(cf[:], coef[:])
            nc.sync.dma_start(sh_sb[:], shm[:])
            d0 = cpool.tile([P4P, 128], F32)
            nc.sync.dma_start(d0[:], dud[:])
            # d0 += K0
            nc.vector.tensor_scalar(d0[:], d0[:], cf[:, 4:5], None, OP.add)
            jp = ctx.enter_context(tc.tile_pool(name="jac", bufs=2))
            sp_p = ctx.enter_context(tc.tile_pool(name="jsp", bufs=2))
            pvp = ctx.enter_context(tc.tile_pool(name="jpv", bufs=2))
            pps = ctx.enter_context(
                tc.tile_pool(name="jps", bufs=2, space="PSUM"))
            d_cur = d0
            g_, dl_, a_, b_ = (cf[:, 0:1], cf[:, 1:2], cf[:, 2:3], cf[:, 3:4])

            def stt(out, in0, scal, in1):
                nc.vector.scalar_tensor_tensor(out, in0, scal, in1,
                                               OP.mult, OP.add)

            def softplus(out_ap, in_ap):
                # Softplus has no ACT table on this build: ln(1 + exp(x)).
                # d stays small (|d| < ~3) so no overflow concerns.
                nc.scalar.activation(out_ap, in_ap, AF.Exp)
                nc.vector.tensor_scalar(out_ap, out_ap, 1.0, None, OP.add)
                nc.scalar.activation(out_ap, out_ap, AF.Ln)

            for it in range(N_JACOBI):
                sp = sp_p.tile([P4P, 128], F32, tag="sp")
                softplus(sp[:], d_cur[:])
                # PV[:, 0:2] = row-shifted d[:, 126:128]; [:, 2:4] = sp ditto
                # (bf16 tail copy so the PE shift-matmul runs in bf16;
                # K padded to 128 with zero rows)
                tl = pvp.tile([128, 4], BF16, tag="tl")
                nc.vector.memset(tl[:], 0.0)
                nc.vector.tensor_copy(tl[0:P4P, 0:2], d_cur[:, 126:128])
                nc.vector.tensor_copy(tl[0:P4P, 2:4], sp[:, 126:128])
                pv_ps = pps.tile([P4P, 4], F32)
                nc.tensor.matmul(pv_ps[:], sh_sb[:], tl[:],
                                 start=True, stop=True, skip_group_check=True)
                pv = pvp.tile([P4P, 4], F32, tag="pv")
                nc.vector.tensor_copy(pv[:], pv_ps[:])
                acc = jp.tile([P4P, 128], F32, tag="acc")
                # interior columns
                stt(acc[:, 1:128], d_cur[:, 0:127], g_, d0[:, 1:128])
                stt(acc[:, 1:128], sp[:, 0:127], dl_, acc[:, 1:128])
                stt(acc[:, 2:128], d_cur[:, 0:126], a_, acc[:, 2:128])
                stt(acc[:, 2:128], sp[:, 0:126], b_, acc[:, 2:128])
                # boundary columns via PV (zero rows at sequence starts)
                stt(acc[:, 0:1], pv[:, 1:2], g_, d0[:, 0:1])
                stt(acc[:, 0:1], pv[:, 3:4], dl_, acc[:, 0:1])
                stt(acc[:, 0:2], pv[:, 0:2], a_, acc[:, 0:2])
                stt(acc[:, 0:2], pv[:, 2:4], b_, acc[:, 0:2])
                if P4_TAPS:
                    nc.sync.dma_start(pvdbg[it], pv[:])
                    nc.sync.dma_start(accdbg[it], acc[:])
                d_cur = acc

            spf = sp_p.tile([P4P, 128], F32, tag="sp")
            softplus(spf[:], d_cur[:])
            lo = cpool.tile([P4P, 128 * 2], F32)
            lov = lo[:].rearrange("p (r x) -> p r x", x=2)
            nc.vector.tensor_scalar(lov[:, :, 0], spf[:], -1.0, None, OP.mult)
            nc.vector.tensor_tensor(lov[:, :, 1], d_cur[:], spf[:],
                                    OP.subtract)
            out_view = outv.rearrange("(p r) x -> p r x", r=128)
            nc.sync.dma_start(out_view, lov)

    nc.compile()
    _BUILD_CACHE[key] = nc
    return nc


# ---------------------------------------------------------------------------
# host-side prep + execution
# ---------------------------------------------------------------------------
def _prep_shared(inputs):
    sh = {}
    for l, (din_, kpad, wkey) in enumerate(((D_IN, 512, "w0"),
                                            (256, 384, "w1"))):
        for d, suf in (("f", ""), ("b", "r")):
            wih = _perm_gates(inputs[f"w_ih_l{l}{suf}"])       # [512, din]
            whh = _perm_gates(inputs[f"w_hh_l{l}{suf}"])       # [512, 128]
            bias = _perm_gates(
                np.asarray(inputs[f"b_ih_l{l}{suf}"])
                + np.asarray(inputs[f"b_hh_l{l}{suf}"]))       # [512]
            aug = np.zeros((kpad, 512), np.float32)
            aug[:din_] = np.asarray(wih, np.float32).T
            aug[din_] = bias
            aug[:, 128:256] *= 2.0     # g-gate prescale: tanh(g)=2*sig(2g)-1
            whhT = np.asarray(whh, np.float32).T
            whhT = whhT.copy()
            whhT[:, 128:256] *= 2.0
            sh[f"{wkey}{d}"] = _bf16(
                aug.reshape(kpad // 128, 128, 512).transpose(1, 0, 2))
            sh[f"whh{l}{d}"] = _bf16(whhT)

    g1, b1 = np.asarray(inputs["bn1_g"]), np.asarray(inputs["bn1_b"])
    m1, v1 = np.asarray(inputs["bn1_m"]), np.asarray(inputs["bn1_v"])
    s1 = g1 / np.sqrt(v1 + EPS)
    t1 = b1 - m1 * s1
    lin_w = np.asarray(inputs["lin_w"])
    LW = lin_w * s1[None, :]
    LB = np.asarray(inputs["lin_b"]) + lin_w @ t1
    g2, b2 = np.asarray(inputs["bn2_g"]), np.asarray(inputs["bn2_b"])
    m2, v2 = np.asarray(inputs["bn2_m"]), np.asarray(inputs["bn2_v"])
    s2 = g2 / np.sqrt(v2 + EPS)
    t2 = b2 - m2 * s2
    out_w, out_b = np.asarray(inputs["out_w"]), np.asarray(inputs["out_b"])
    W1, W2, W3 = out_w[:, 0:2], out_w[:, 2:4], out_w[:, 4:68]
    w3d = W3[1] - W3[0]
    K0 = (out_b[1] - out_b[0]) + t2 @ w3d
    w1d, w2d = W1[1] - W1[0], W2[1] - W2[0]
    alpha, beta = w1d[1], w1d[0] + w1d[1]
    gamma, delta = w2d[1], w2d[0] + w2d[1]

    sh["lwk"] = _bf16(LW.T.reshape(2, 128, 64).transpose(1, 0, 2))
    sh["lbv"] = np.asarray(LB, np.float32).reshape(64, 1)
    sh["w3s"] = _bf16((w3d * s2).reshape(64, 1))
    QB = max(T // 128, 1)
    P4P = BL * QB
    coefs = np.zeros((P4P, 8), np.float32)
    coefs[:, 0] = gamma
    coefs[:, 1] = -delta
    coefs[:, 2] = alpha
    coefs[:, 3] = -beta
    coefs[:, 4] = K0
    sh["coef"] = coefs
    shmat = np.zeros((128, P4P), np.float32)
    shmat[:P4P] = np.eye(P4P, k=1, dtype=np.float32)
    shmat[:, ::QB] = 0.0        # zero columns at sequence starts
    sh["shm"] = _bf16(shmat)
    sh["ident"] = _bf16(np.eye(128, dtype=np.float32))
    return sh


def _prep_core(x_core):
    # x_core: [BL, T, 400] -> aug kxn [128, 4, T*BL] bf16, b-major cols
    xt = np.zeros((512, T * BL), np.float32)
    xt[:D_IN] = np.asarray(x_core, np.float32).transpose(2, 0, 1).reshape(
        D_IN, T * BL)
    xt[D_IN] = 1.0
    return _bf16(xt.reshape(4, 128, T * BL).transpose(1, 0, 2))


def kernel(**inputs):
    nc = _build()
    sh = _prep_shared(inputs)
    x = np.asarray(inputs["x"], np.float32)
    in_maps = []
    for cidx in range(N_CORES):
        m = dict(sh)
        m["xk"] = _prep_core(x[cidx * BL:(cidx + 1) * BL])
        in_maps.append(m)
    res = run_bass_kernel_spmd(nc, in_maps, list(range(N_CORES)))
    outs = [np.asarray(res.results[i]["outv"], np.float32)
            for i in range(N_CORES)]
    return np.concatenate(outs, axis=0)


if __name__ == "__main__":
    import time
    t0 = time.time()
    print(f"building T={T}...")
    _build()
    print(f"built in {time.time() - t0:.1f}s")
